# revision 1
# baseline (speedup 1.0000x reference)
"""Causal GQA attention (S=2048, B=2, HQ=32, HKV=8, D=128) on 8 trn2 cores.

Sharding: the 16 (batch, kv-head) pairs are split 2 per core (data+head
parallel). Each pair carries group=4 query heads -> 8 attention heads/core.

Device kernel computes, per head, S^T = (Q K^T)^T in PSUM chunk-by-chunk
(so the softmax free axis never needs an on-chip transpose), exponentiates
on ACT into SBUF (P^T), applies the causal triangular mask only on the
128x128 diagonal block, then accumulates out^T = V^T-style matmuls with V
stationary and the softmax denominators with a ones-column matmul. All
matmul operands are viewed as float32r (full-rate fp32 on the PE array for
moving dim >= 256).

Host side only re-lays-out data: Q/K are fed pre-transposed [d, s], V as
[k_local, ktile, d], and the returned out^T [d, s] is transposed back.
"""

import numpy as np

import concourse.bass as bass
import concourse.mybir as mybir
import concourse.tile as tile
from concourse import bacc, bass_utils
from concourse.masks import make_upper_triangular

S, B, HQ, HKV, D = 2048, 2, 32, 8, 128
G = HQ // HKV                      # 4 query heads per kv head
NCORES = 8
NPAIRS = B * HKV                   # 16 (batch, kv-head) pairs
PAIRS_PER_CORE = NPAIRS // NCORES  # 2
HEADS_PER_CORE = PAIRS_PER_CORE * G  # 8
SCALE = 1.0 / float(np.sqrt(D))
QC = 512                           # q-chunk (PSUM bank) width
NQC = S // QC                      # 4
KT = 128                           # k-tile (partition) width
NKT = S // KT                      # 16

F32 = mybir.dt.float32
F32R = mybir.dt.float32r
BF16 = mybir.dt.bfloat16


def emit_core_program(tc, qt, kt, v, recd, ot):
    """Emit the per-core program.

    qt: [HEADS_PER_CORE, D, S] f32r   Q^T per head ([d, q])
    kt: [PAIRS_PER_CORE, D, S] f32r   K^T per pair ([d, k])
    v:  [PAIRS_PER_CORE, 128, NKT*D] f32  V per pair ([k_local, kt, d])
    recd: [HEADS_PER_CORE, NQC, QC] f32 DRAM scratch for 1/sum rows
    ot: [HEADS_PER_CORE, D, S] f32   out^T per head ([d, q])

    QK^T runs in float32r (full-rate fp32); the P*V side runs in bf16
    (P in [0, e^~5], V order-1: bf16 keeps ~4e-3 relative accuracy and the
    softmax normalization cancels much of the P rounding).
    """
    from contextlib import ExitStack

    nc = tc.nc
    with ExitStack() as ctx:
        _emit_core_program(ctx, tc, nc, qt, kt, v, recd, ot)


def _emit_core_program(ctx, tc, nc, qt, kt, v, recd, ot):
    singles = ctx.enter_context(tc.tile_pool(name="singles", bufs=1))
    kv_pool = ctx.enter_context(tc.tile_pool(name="kv", bufs=2))
    q_pool = ctx.enter_context(tc.tile_pool(name="q", bufs=2))
    pt_pool = ctx.enter_context(tc.tile_pool(name="pt", bufs=3))
    ob_pool = ctx.enter_context(tc.tile_pool(name="ob", bufs=3))
    nrm_pool = ctx.enter_context(tc.tile_pool(name="nrm", bufs=3))
    ps_s = ctx.enter_context(tc.tile_pool(name="ps_s", bufs=1, space="PSUM"))
    ps_o = ctx.enter_context(tc.tile_pool(name="ps_o", bufs=5, space="PSUM"))
    ps_sum = ctx.enter_context(tc.tile_pool(name="ps_sum", bufs=1, space="PSUM"))

    # Constants
    # tri[k, q] = 1.0 where q >= k (allowed), 0.0 where q < k. Multiplied
    # into the P^T diagonal block after exp (bf16, off the QK->exp path).
    trif = singles.tile([128, 128], F32)
    make_upper_triangular(nc, trif[:], val=1.0, diag=True)
    tri = singles.tile([128, 128], BF16)
    nc.scalar.copy(out=tri[:], in_=trif[:])
    onesc = singles.tile([128, 1], BF16)   # ones column (sum-over-k lhsT)
    nc.vector.memset(onesc[:], 1.0)

    for pair in range(PAIRS_PER_CORE):
        kt_sb = kv_pool.tile([D, S], F32R, tag="kt")
        nc.sync.dma_start(out=kt_sb[:], in_=kt[pair])
        v_sb = kv_pool.tile([128, NKT * D], BF16, tag="v")
        nc.gpsimd.dma_start(out=v_sb[:], in_=v[pair])  # casting DMA f32->bf16

        for g in range(G):
            head = pair * G + g
            q_sb = q_pool.tile([D, S], F32R)
            nc.sync.dma_start(out=q_sb[:], in_=qt[head])

            s_ps = ps_s.tile([128, 2 * QC], F32)    # 2 banks of S^T staging
            # out^T accumulators: one PSUM bank per q-chunk, rotating through
            # 5 banks so the next head's chunk never WARs on this head's
            # in-flight normalization
            o_tiles = [ps_o.tile([128, QC], F32, tag="o", name=f"o_{head}_{c}")
                       for c in range(NQC)]
            sum_ps = ps_sum.tile([128, QC], F32)    # 1 bank: chunk c at row 32c

            norm_state = {}

            def norm_stage_a(c):
                # sums row PSUM -> SBUF (ACT), then DMA-reshape to [128, 4]
                # so the reciprocal runs 128 lanes wide
                row = slice(32 * c, 32 * c + 1)
                sr = nrm_pool.tile([128, QC], F32, tag="sumrow")
                nc.scalar.copy(out=sr[row, :], in_=sum_ps[row, :])
                srec = nrm_pool.tile([128, NQC], F32, tag="srec")
                nc.sync.dma_start(out=srec[:], in_=sr[row, :])
                norm_state[c] = srec

            def norm_stage_b(c):
                srec = norm_state[c]
                srec2 = nrm_pool.tile([128, NQC], F32, tag="srec2")
                nc.vector.reciprocal(out=srec2[:], in_=srec[:])
                nc.sync.dma_start(out=recd[head, c], in_=srec2[:])
                bcs = nrm_pool.tile([128, QC], F32, tag="bc")
                nc.sync.dma_start(
                    out=bcs[:], in_=recd[head, c].partition_broadcast(128))
                norm_state[c] = bcs

            def norm_stage_c(c):
                bcs = norm_state.pop(c)
                osb = ob_pool.tile([128, QC], F32)
                nc.vector.tensor_mul(osb[:], o_tiles[c][:], bcs[:])
                nc.sync.dma_start(
                    out=ot[head][:, QC * c:QC * (c + 1)], in_=osb[:])

            for kti in range(NKT):
                w = KT * kti          # first allowed q for this k-tile
                c0 = w // QC          # first overlapping q-chunk
                p_kt = pt_pool.tile([128, S], BF16)  # P^T rows for this k-tile

                def s_slice(c):
                    off = max(0, w - QC * c)
                    base = QC * ((c - c0) % 2)
                    return off, s_ps[:, base + off:base + QC]

                def av_ones(c):
                    off = max(0, w - QC * c)
                    rhs = p_kt[:, QC * c + off:QC * (c + 1)]
                    first = kti == 0
                    last = kti == 4 * c + 3
                    nc.tensor.matmul(
                        out=o_tiles[c][:, off:QC],
                        lhsT=v_sb[:, D * kti:D * (kti + 1)],
                        rhs=rhs, start=first, stop=last,
                    )
                    nc.tensor.matmul(
                        out=sum_ps[32 * c:32 * c + 1, off:QC],
                        lhsT=onesc[:],
                        rhs=rhs, start=first, stop=last,
                        tile_position=(0, 32 * c),
                    )

                # interleave QK -> exp -> (prev chunk AV) so PE always has a
                # runnable matmul while ACT exponentiates
                prev = None
                for c in range(c0, NQC):
                    off, s_ap = s_slice(c)
                    nc.tensor.matmul(
                        out=s_ap,
                        lhsT=kt_sb[:, w:w + KT],
                        rhs=q_sb[:, QC * c + off:QC * (c + 1)],
                        start=True, stop=True,
                    )
                    nc.scalar.activation(
                        p_kt[:, QC * c + off:QC * (c + 1)], s_ap,
                        mybir.ActivationFunctionType.Exp, scale=SCALE)
                    if c == c0:
                        # causal mask: zero q < k on the diagonal block
                        nc.vector.tensor_mul(
                            p_kt[:, w:w + KT], p_kt[:, w:w + KT], tri[:])
                    if prev is not None:
                        av_ones(prev)
                    prev = c
                av_ones(prev)

                # Normalization, software-pipelined across k-tile iterations
                # so the slow partition-broadcast DMA never blocks the DVE
                # stream: chunk c finishes accumulating at kti=4c+3 (stage A:
                # pull sums row + reshape), recip + broadcast issue at 4c+4
                # (stage B), multiply + store at 4c+5 (stage C).
                if kti >= 3 and (kti - 3) % 4 == 0:
                    norm_stage_a((kti - 3) // 4)
                if kti >= 4 and (kti - 4) % 4 == 0:
                    norm_stage_b((kti - 4) // 4)
                if kti >= 5 and (kti - 5) % 4 == 0:
                    norm_stage_c((kti - 5) // 4)

            # drain chunk 3 (finished at kti=15)
            norm_stage_b(3)
            norm_stage_c(3)


_CACHED_NC = None


def build_program():
    global _CACHED_NC
    if _CACHED_NC is not None:
        return _CACHED_NC
    nc = bacc.Bacc("TRN2", target_bir_lowering=False, debug=False,
                   num_devices=NCORES)
    qt = nc.dram_tensor("qt", [HEADS_PER_CORE, D, S], F32R,
                        kind="ExternalInput").ap()
    kt = nc.dram_tensor("kt", [PAIRS_PER_CORE, D, S], F32R,
                        kind="ExternalInput").ap()
    v = nc.dram_tensor("v", [PAIRS_PER_CORE, 128, NKT * D], F32,
                       kind="ExternalInput").ap()
    recd = nc.dram_tensor("recd", [HEADS_PER_CORE, NQC, QC], F32,
                          kind="Internal").ap()
    ot = nc.dram_tensor("ot", [HEADS_PER_CORE, D, S], F32,
                        kind="ExternalOutput").ap()
    with tile.TileContext(nc) as tc:
        emit_core_program(tc, qt, kt, v, recd, ot)
    nc.compile()
    _CACHED_NC = nc
    return nc


def shard_inputs(query, key, value):
    """Full inputs -> list of 8 per-core in_maps (host-side relayout only)."""
    query = np.asarray(query, dtype=np.float32)
    key = np.asarray(key, dtype=np.float32)
    value = np.asarray(value, dtype=np.float32)

    # Q: [S,B,HQ,D] -> [B*HKV, G, D, S]
    qtall = np.ascontiguousarray(
        query.reshape(S, B, HKV, G, D).transpose(1, 2, 3, 4, 0)
    ).reshape(NPAIRS, G, D, S)
    # K: [S,B,HKV,D] -> [B*HKV, D, S]
    ktall = np.ascontiguousarray(
        key.transpose(1, 2, 3, 0)).reshape(NPAIRS, D, S)
    # V: [S,B,HKV,D] -> [B*HKV, k_local=128, NKT*D]
    vall = np.ascontiguousarray(
        value.reshape(NKT, 128, B, HKV, D).transpose(2, 3, 1, 0, 4)
    ).reshape(NPAIRS, 128, NKT * D)

    in_maps = []
    for c in range(NCORES):
        p0 = PAIRS_PER_CORE * c
        p1 = p0 + PAIRS_PER_CORE
        in_maps.append({
            "qt": np.ascontiguousarray(qtall[p0:p1].reshape(HEADS_PER_CORE, D, S)),
            "kt": np.ascontiguousarray(ktall[p0:p1]),
            "v": np.ascontiguousarray(vall[p0:p1]),
        })
    return in_maps


def unshard_output(results):
    """8 per-core {'ot': [8, D, S]} -> full [S, B, HQ, D]."""
    ot = np.stack([r["ot"] for r in results])          # [8, 8, D, S]
    ot = ot.reshape(B, HKV, G, D, S)                   # pairs major -> b, hkv
    out = np.ascontiguousarray(ot.transpose(4, 0, 1, 2, 3))  # [S,B,HKV,G,D]
    return out.reshape(S, B, HQ, D)


def kernel(query, key, value, _trace=False, _return_bkr=False):
    nc = build_program()
    in_maps = shard_inputs(query, key, value)
    bkr = bass_utils.run_bass_kernel_spmd(
        nc, in_maps, core_ids=list(range(NCORES)), trace=_trace)
    out = unshard_output(bkr.results)
    if _return_bkr:
        return out, bkr
    return out


if __name__ == "__main__":
    q = np.random.randn(S, B, HQ, D).astype(np.float32)
    k = np.random.randn(S, B, HKV, D).astype(np.float32)
    vv = np.random.randn(S, B, HKV, D).astype(np.float32)
    o = kernel(q, k, vv)
    print("out", o.shape, o.dtype, float(np.abs(o).max()))



# revision 5
# speedup vs baseline: 1.1835x; 1.1835x over previous
"""Causal GQA attention (S=2048, B=2, HQ=32, HKV=8, D=128) on 8 trn2 cores.

Sharding: the 16 (batch, kv-head) pairs are split 2 per core (data+head
parallel). Each pair carries group=4 query heads -> 8 attention heads/core.

Device kernel computes, per head, S^T = (Q K^T)^T in PSUM chunk-by-chunk
(so the softmax free axis never needs an on-chip transpose), exponentiates
on ACT into SBUF (P^T), applies the causal triangular mask only on the
128x128 diagonal block, then accumulates out^T = V^T-style matmuls with V
stationary and the softmax denominators with a ones-column matmul. All
matmul operands are viewed as float32r (full-rate fp32 on the PE array for
moving dim >= 256).

Host side only re-lays-out data: Q/K are fed pre-transposed [d, s], V as
[k_local, ktile, d], and the returned out^T [d, s] is transposed back.
"""

import numpy as np

import concourse.bass as bass
import concourse.mybir as mybir
import concourse.tile as tile
from concourse import bacc, bass_utils
from concourse.masks import make_upper_triangular

S, B, HQ, HKV, D = 2048, 2, 32, 8, 128
G = HQ // HKV                      # 4 query heads per kv head
NCORES = 8
NPAIRS = B * HKV                   # 16 (batch, kv-head) pairs
PAIRS_PER_CORE = NPAIRS // NCORES  # 2
HEADS_PER_CORE = PAIRS_PER_CORE * G  # 8
SCALE = 1.0 / float(np.sqrt(D))
QC = 512                           # q-chunk (PSUM bank) width
NQC = S // QC                      # 4
KT = 128                           # k-tile (partition) width
NKT = S // KT                      # 16

F32 = mybir.dt.float32
F32R = mybir.dt.float32r
BF16 = mybir.dt.bfloat16


def emit_core_program(tc, qt, kt, v, recd, ot):
    """Emit the per-core program.

    qt: [HEADS_PER_CORE, D, S] bf16   Q^T per head ([d, q])
    kt: [PAIRS_PER_CORE, D, S] bf16   K^T per pair ([d, k])
    v:  [PAIRS_PER_CORE, 128, NKT*D] bf16  V per pair ([k_local, kt, d])
    recd: [HEADS_PER_CORE, NQC, QC] f32 DRAM scratch for 1/sum rows
    ot: [HEADS_PER_CORE, D, S] f32   out^T per head ([d, q])

    Everything on the PE runs in bf16: fp32/fp32r matmuls double-pump the
    array and keep PE activity below the HAM threshold, so the clock sits
    at 1.2 GHz for most of the kernel. bf16 streams 1 col/cycle, keeps the
    array dense (HAM stays at K=8/8), and softmax normalization cancels
    most of the rounding (measured ~2.4e-3 rel err, same as fp32r QK).
    """
    from contextlib import ExitStack

    nc = tc.nc
    with ExitStack() as ctx:
        _emit_core_program(ctx, tc, nc, qt, kt, v, recd, ot)


def _emit_core_program(ctx, tc, nc, qt, kt, v, recd, ot):
    singles = ctx.enter_context(tc.tile_pool(name="singles", bufs=1))
    kv_pool = ctx.enter_context(tc.tile_pool(name="kv", bufs=2))
    q_pool = ctx.enter_context(tc.tile_pool(name="q", bufs=2))
    pt_pool = ctx.enter_context(tc.tile_pool(name="pt", bufs=3))
    ob_pool = ctx.enter_context(tc.tile_pool(name="ob", bufs=3))
    nrm_pool = ctx.enter_context(tc.tile_pool(name="nrm", bufs=3))
    ps_s = ctx.enter_context(tc.tile_pool(name="ps_s", bufs=1, space="PSUM"))
    ps_o = ctx.enter_context(tc.tile_pool(name="ps_o", bufs=5, space="PSUM"))
    ps_sum = ctx.enter_context(tc.tile_pool(name="ps_sum", bufs=1, space="PSUM"))

    # Constants
    # tri[k, q] = 1.0 where q >= k (allowed), 0.0 where q < k. Multiplied
    # into the P^T diagonal block after exp (bf16, off the QK->exp path).
    trif = singles.tile([128, 128], F32)
    make_upper_triangular(nc, trif[:], val=1.0, diag=True)
    tri = singles.tile([128, 128], BF16)
    nc.scalar.copy(out=tri[:], in_=trif[:])
    onesc = singles.tile([128, 1], BF16)   # ones column (sum-over-k lhsT)
    nc.vector.memset(onesc[:], 1.0)

    for pair in range(PAIRS_PER_CORE):
        kt_sb = kv_pool.tile([D, S], BF16, tag="kt")
        nc.sync.dma_start(out=kt_sb[:], in_=kt[pair])
        v_sb = kv_pool.tile([128, NKT * D], BF16, tag="v")
        nc.sync.dma_start(out=v_sb[:], in_=v[pair])

        for g in range(G):
            head = pair * G + g
            q_sb = q_pool.tile([D, S], BF16)
            nc.sync.dma_start(out=q_sb[:], in_=qt[head])

            s_ps = ps_s.tile([128, 2 * QC], F32)    # 2 banks of S^T staging
            # out^T accumulators: one PSUM bank per q-chunk, rotating through
            # 5 banks so the next head's chunk never WARs on this head's
            # in-flight normalization
            o_tiles = [ps_o.tile([128, QC], F32, tag="o", name=f"o_{head}_{c}")
                       for c in range(NQC)]
            sum_ps = ps_sum.tile([128, QC], F32)    # 1 bank: chunk c at row 32c

            norm_state = {}

            def norm_stage_a(c):
                # sums row PSUM -> SBUF (ACT), then DMA-reshape to [128, 4]
                # so the reciprocal runs 128 lanes wide
                row = slice(32 * c, 32 * c + 1)
                sr = nrm_pool.tile([128, QC], F32, tag="sumrow")
                nc.scalar.copy(out=sr[row, :], in_=sum_ps[row, :])
                srec = nrm_pool.tile([128, NQC], F32, tag="srec")
                nc.sync.dma_start(out=srec[:], in_=sr[row, :])
                norm_state[c] = srec

            def norm_stage_b(c):
                srec = norm_state[c]
                srec2 = nrm_pool.tile([128, NQC], F32, tag="srec2")
                nc.vector.reciprocal(out=srec2[:], in_=srec[:])
                nc.sync.dma_start(out=recd[head, c], in_=srec2[:])
                bcs = nrm_pool.tile([128, QC], F32, tag="bc")
                nc.sync.dma_start(
                    out=bcs[:], in_=recd[head, c].partition_broadcast(128))
                norm_state[c] = bcs

            def norm_stage_c(c):
                bcs = norm_state.pop(c)
                osb = ob_pool.tile([128, QC], F32)
                nc.vector.tensor_mul(osb[:], o_tiles[c][:], bcs[:])
                nc.sync.dma_start(
                    out=ot[head][:, QC * c:QC * (c + 1)], in_=osb[:])

            for kti in range(NKT):
                w = KT * kti          # first allowed q for this k-tile
                c0 = w // QC          # first overlapping q-chunk
                p_kt = pt_pool.tile([128, S], BF16)  # P^T rows for this k-tile

                def s_slice(c):
                    off = max(0, w - QC * c)
                    base = QC * ((c - c0) % 2)
                    return off, s_ps[:, base + off:base + QC]

                def av_ones(c):
                    off = max(0, w - QC * c)
                    rhs = p_kt[:, QC * c + off:QC * (c + 1)]
                    first = kti == 0
                    last = kti == 4 * c + 3
                    nc.tensor.matmul(
                        out=o_tiles[c][:, off:QC],
                        lhsT=v_sb[:, D * kti:D * (kti + 1)],
                        rhs=rhs, start=first, stop=last,
                    )
                    nc.tensor.matmul(
                        out=sum_ps[32 * c:32 * c + 1, off:QC],
                        lhsT=onesc[:],
                        rhs=rhs, start=first, stop=last,
                        tile_position=(0, 32 * c),
                    )

                # interleave QK -> exp -> (prev chunk AV) so PE always has a
                # runnable matmul while ACT exponentiates
                prev = None
                for c in range(c0, NQC):
                    off, s_ap = s_slice(c)
                    nc.tensor.matmul(
                        out=s_ap,
                        lhsT=kt_sb[:, w:w + KT],
                        rhs=q_sb[:, QC * c + off:QC * (c + 1)],
                        start=True, stop=True,
                    )
                    nc.scalar.activation(
                        p_kt[:, QC * c + off:QC * (c + 1)], s_ap,
                        mybir.ActivationFunctionType.Exp, scale=SCALE)
                    if c == c0:
                        # causal mask: zero q < k on the diagonal block
                        nc.vector.tensor_mul(
                            p_kt[:, w:w + KT], p_kt[:, w:w + KT], tri[:])
                    if prev is not None:
                        av_ones(prev)
                    prev = c
                av_ones(prev)

                # Normalization, software-pipelined across k-tile iterations
                # so the slow partition-broadcast DMA never blocks the DVE
                # stream: chunk c finishes accumulating at kti=4c+3 (stage A:
                # pull sums row + reshape), recip + broadcast issue at 4c+4
                # (stage B), multiply + store at 4c+5 (stage C).
                if kti >= 3 and (kti - 3) % 4 == 0:
                    norm_stage_a((kti - 3) // 4)
                if kti >= 4 and (kti - 4) % 4 == 0:
                    norm_stage_b((kti - 4) // 4)
                if kti >= 5 and (kti - 5) % 4 == 0:
                    norm_stage_c((kti - 5) // 4)

            # drain chunk 3 (finished at kti=15)
            norm_stage_b(3)
            norm_stage_c(3)


_CACHED_NC = None


def build_program():
    global _CACHED_NC
    if _CACHED_NC is not None:
        return _CACHED_NC
    nc = bacc.Bacc("TRN2", target_bir_lowering=False, debug=False,
                   num_devices=NCORES)
    qt = nc.dram_tensor("qt", [HEADS_PER_CORE, D, S], BF16,
                        kind="ExternalInput").ap()
    kt = nc.dram_tensor("kt", [PAIRS_PER_CORE, D, S], BF16,
                        kind="ExternalInput").ap()
    v = nc.dram_tensor("v", [PAIRS_PER_CORE, 128, NKT * D], BF16,
                       kind="ExternalInput").ap()
    recd = nc.dram_tensor("recd", [HEADS_PER_CORE, NQC, QC], F32,
                          kind="Internal").ap()
    ot = nc.dram_tensor("ot", [HEADS_PER_CORE, D, S], F32,
                        kind="ExternalOutput").ap()
    with tile.TileContext(nc) as tc:
        emit_core_program(tc, qt, kt, v, recd, ot)
    nc.compile()
    _CACHED_NC = nc
    return nc


def shard_inputs(query, key, value):
    """Full inputs -> list of 8 per-core in_maps (host-side relayout + bf16
    cast; halves the HBM input traffic and keeps the PE in bf16)."""
    import ml_dtypes
    bf16 = ml_dtypes.bfloat16
    query = np.asarray(query, dtype=np.float32).astype(bf16)
    key = np.asarray(key, dtype=np.float32).astype(bf16)
    value = np.asarray(value, dtype=np.float32).astype(bf16)

    # Q: [S,B,HQ,D] -> [B*HKV, G, D, S]
    qtall = np.ascontiguousarray(
        query.reshape(S, B, HKV, G, D).transpose(1, 2, 3, 4, 0)
    ).reshape(NPAIRS, G, D, S)
    # K: [S,B,HKV,D] -> [B*HKV, D, S]
    ktall = np.ascontiguousarray(
        key.transpose(1, 2, 3, 0)).reshape(NPAIRS, D, S)
    # V: [S,B,HKV,D] -> [B*HKV, k_local=128, NKT*D]
    vall = np.ascontiguousarray(
        value.reshape(NKT, 128, B, HKV, D).transpose(2, 3, 1, 0, 4)
    ).reshape(NPAIRS, 128, NKT * D)

    in_maps = []
    for c in range(NCORES):
        p0 = PAIRS_PER_CORE * c
        p1 = p0 + PAIRS_PER_CORE
        in_maps.append({
            "qt": np.ascontiguousarray(qtall[p0:p1].reshape(HEADS_PER_CORE, D, S)),
            "kt": np.ascontiguousarray(ktall[p0:p1]),
            "v": np.ascontiguousarray(vall[p0:p1]),
        })
    return in_maps


def unshard_output(results):
    """8 per-core {'ot': [8, D, S]} -> full [S, B, HQ, D]."""
    ot = np.stack([r["ot"] for r in results])          # [8, 8, D, S]
    ot = ot.reshape(B, HKV, G, D, S)                   # pairs major -> b, hkv
    out = np.ascontiguousarray(ot.transpose(4, 0, 1, 2, 3))  # [S,B,HKV,G,D]
    return out.reshape(S, B, HQ, D)


def kernel(query, key, value, _trace=False, _return_bkr=False):
    nc = build_program()
    in_maps = shard_inputs(query, key, value)
    bkr = bass_utils.run_bass_kernel_spmd(
        nc, in_maps, core_ids=list(range(NCORES)), trace=_trace)
    out = unshard_output(bkr.results)
    if _return_bkr:
        return out, bkr
    return out


if __name__ == "__main__":
    q = np.random.randn(S, B, HQ, D).astype(np.float32)
    k = np.random.randn(S, B, HKV, D).astype(np.float32)
    vv = np.random.randn(S, B, HKV, D).astype(np.float32)
    o = kernel(q, k, vv)
    print("out", o.shape, o.dtype, float(np.abs(o).max()))



# revision 8
# speedup vs baseline: 1.6046x; 1.3558x over previous
"""Causal GQA attention (S=2048, B=2, HQ=32, HKV=8, D=128) on 8 trn2 cores.

Sharding: the 16 (batch, kv-head) pairs are split 2 per core (data+head
parallel). Each pair carries group=4 query heads -> 8 attention heads/core.

Per head the kernel runs flash-attention style with the q-chunk loop OUTER
and the k-tile loop INNER:

  for q-chunk c (512 wide):                 # o accumulates in ONE psum bank
    for k-tile pair (t0, t1):               # 128-row k tiles, 2 at a time
      S^T(t0), S^T(t1) = (K_t Q_c^T) into a 2-bank psum pair
      P^T pair = exp(S^T pair)              # one 1024-wide ACT instruction
      tri-mask diagonal blocks (DVE)
      T += P^T tiles (DVE, bf16)            # cross-k-tile accumulation
      o += V_t^T P^T(t0), V_t^T P^T(t1)     # PE, accumulate in one bank
    den row = ones^T T                      # ONE 512-col matmul per chunk
    out_c = o * (1/den broadcast)           # DVE + DMA round trip

Everything on the PE runs bf16 (fp32/fp32r matmuls double-pump the array
and trip the power throttler). The one-matmul-per-chunk denominator (vs
one per k-tile) cuts PE streaming by ~1/3; the paired exp halves the
~185ns-per-instruction ACT bubble. Output is stored bf16 and upcast on
host (measured ~4e-3 rel err overall vs the 2e-2 gate).

Host side only re-lays-out data: Q/K are fed pre-transposed [d, s] bf16,
V as [k_local, ktile, d] bf16, and the returned out^T [d, s] bf16 is
transposed back and upcast.
"""

import numpy as np

import concourse.bass as bass
import concourse.mybir as mybir
import concourse.tile as tile
from concourse import bacc, bass_utils
from concourse.masks import make_upper_triangular

S, B, HQ, HKV, D = 2048, 2, 32, 8, 128
G = HQ // HKV                      # 4 query heads per kv head
NCORES = 8
NPAIRS = B * HKV                   # 16 (batch, kv-head) pairs
PAIRS_PER_CORE = NPAIRS // NCORES  # 2
HEADS_PER_CORE = PAIRS_PER_CORE * G  # 8
SCALE = 1.0 / float(np.sqrt(D))
QC = 512                           # q-chunk (PSUM bank) width
NQC = S // QC                      # 4
KT = 128                           # k-tile (partition) width
NKT = S // KT                      # 16

F32 = mybir.dt.float32
BF16 = mybir.dt.bfloat16


def emit_core_program(tc, qt, kt, v, recd, ot):
    """Emit the per-core program.

    qt: [HEADS_PER_CORE, D, S] bf16   Q^T per head ([d, q])
    kt: [PAIRS_PER_CORE, D, S] bf16   K^T per pair ([d, k])
    v:  [PAIRS_PER_CORE, 128, NKT*D] bf16  V per pair ([k_local, kt, d])
    recd: [HEADS_PER_CORE, NQC, QC] f32 DRAM scratch for 1/sum rows
    ot: [HEADS_PER_CORE, D, S] bf16  out^T per head ([d, q])
    """
    from contextlib import ExitStack

    nc = tc.nc
    with ExitStack() as ctx:
        _emit_core_program(ctx, tc, nc, qt, kt, v, recd, ot)


def _emit_core_program(ctx, tc, nc, qt, kt, v, recd, ot):
    singles = ctx.enter_context(tc.tile_pool(name="singles", bufs=1))
    kv_pool = ctx.enter_context(tc.tile_pool(name="kv", bufs=2))
    q_pool = ctx.enter_context(tc.tile_pool(name="q", bufs=2))
    pp_pool = ctx.enter_context(tc.tile_pool(name="pp", bufs=4))
    t_pool = ctx.enter_context(tc.tile_pool(name="tt", bufs=2))
    ob_pool = ctx.enter_context(tc.tile_pool(name="ob", bufs=3))
    nrm_pool = ctx.enter_context(tc.tile_pool(name="nrm", bufs=3))
    ps_pair = ctx.enter_context(tc.tile_pool(name="ps_pair", bufs=2, space="PSUM"))
    ps_o = ctx.enter_context(tc.tile_pool(name="ps_o", bufs=3, space="PSUM"))
    ps_sum = ctx.enter_context(tc.tile_pool(name="ps_sum", bufs=1, space="PSUM"))

    # Constants
    # tri[k, q] = 1.0 where q >= k (allowed), 0.0 where q < k. Multiplied
    # into the P^T diagonal blocks after exp.
    trif = singles.tile([128, 128], F32)
    make_upper_triangular(nc, trif[:], val=1.0, diag=True)
    tri = singles.tile([128, 128], BF16)
    nc.scalar.copy(out=tri[:], in_=trif[:])
    onesc = singles.tile([128, 1], BF16)   # ones column (sum-over-k lhsT)
    nc.vector.memset(onesc[:], 1.0)

    # Deferred normalization stages: one closure is popped and emitted at
    # the top of each k-tile-pair iteration, so the slow DMA round trips
    # (recip row -> DRAM -> partition-broadcast) never head-of-line block
    # the DVE queue that feeds T accumulation.
    deferred = []

    def pop_deferred():
        if deferred:
            deferred.pop(0)()

    exp = mybir.ActivationFunctionType.Exp

    for pair in range(PAIRS_PER_CORE):
        kt_sb = kv_pool.tile([D, S], BF16, tag="kt")
        nc.sync.dma_start(out=kt_sb[:], in_=kt[pair])
        v_sb = kv_pool.tile([128, NKT * D], BF16, tag="v")
        nc.gpsimd.dma_start(out=v_sb[:], in_=v[pair])

        for g in range(G):
            head = pair * G + g
            q_sb = q_pool.tile([D, S], BF16)
            nc.sync.dma_start(out=q_sb[:], in_=qt[head])

            for c in range(NQC):
                ntiles = 4 * c + 4
                npairs = ntiles // 2
                qs = q_sb[:, QC * c:QC * (c + 1)]
                o_ps = ps_o.tile([128, QC], F32, tag="o", name=f"o_{head}_{c}")
                tt = t_pool.tile([128, QC], BF16, tag="T", name=f"T_{head}_{c}")

                for j in range(npairs):
                    pop_deferred()
                    t0, t1 = 2 * j, 2 * j + 1
                    o0 = max(0, KT * t0 - QC * c)
                    o1 = max(0, KT * t1 - QC * c)
                    sp = ps_pair.tile([128, 2 * QC], F32, tag="spair")
                    pp = pp_pool.tile([128, 2 * QC], BF16, tag="pp")

                    # S^T = (Q K^T)^T for both k-tiles of the pair
                    nc.tensor.matmul(
                        out=sp[:, o0:QC],
                        lhsT=kt_sb[:, KT * t0:KT * (t0 + 1)],
                        rhs=qs[:, o0:QC], start=True, stop=True)
                    nc.tensor.matmul(
                        out=sp[:, QC + o1:2 * QC],
                        lhsT=kt_sb[:, KT * t1:KT * (t1 + 1)],
                        rhs=qs[:, o1:QC], start=True, stop=True)

                    # one wide exp over the pair (cols [QC+o0 : QC+o1] are
                    # junk from the staging bank; nothing consumes them)
                    nc.scalar.activation(
                        pp[:, o0:2 * QC], sp[:, o0:2 * QC], exp, scale=SCALE)

                    # causal mask on the diagonal 128x128 blocks
                    for tti, oo, base in ((t0, o0, 0), (t1, o1, QC)):
                        if tti >= 4 * c:
                            blk = slice(base + oo, base + oo + KT)
                            nc.vector.tensor_mul(pp[:, blk], pp[:, blk], tri[:])

                    # T accumulation (bf16, 2x DVE mode)
                    if j == 0:
                        nc.vector.tensor_copy(tt[:, :], pp[:, 0:QC])
                    else:
                        nc.vector.tensor_add(
                            tt[:, o0:QC], tt[:, o0:QC], pp[:, o0:QC])
                    nc.vector.tensor_add(
                        tt[:, o1:QC], tt[:, o1:QC], pp[:, QC + o1:2 * QC])

                    # out^T accumulation, V stationary
                    nc.tensor.matmul(
                        out=o_ps[:, o0:QC],
                        lhsT=v_sb[:, D * t0:D * (t0 + 1)],
                        rhs=pp[:, o0:QC],
                        start=(j == 0), stop=False)
                    nc.tensor.matmul(
                        out=o_ps[:, o1:QC],
                        lhsT=v_sb[:, D * t1:D * (t1 + 1)],
                        rhs=pp[:, QC + o1:2 * QC],
                        start=False, stop=(j == npairs - 1))

                # denominator: one 512-col stream over the accumulated T
                sum_ps = ps_sum.tile([128, QC], F32, tag="sum")
                nc.tensor.matmul(
                    out=sum_ps[0:1, :], lhsT=onesc[:], rhs=tt[:, :],
                    start=True, stop=True)

                def stage_b(head=head, c=c, sum_ps=sum_ps):
                    rrow = nrm_pool.tile([1, QC], F32, tag="rrow")
                    nc.vector.reciprocal(out=rrow[:], in_=sum_ps[0:1, :])
                    nc.sync.dma_start(out=recd[head, c], in_=rrow[:])
                    bcs = nrm_pool.tile([128, QC], F32, tag="bc")
                    nc.sync.dma_start(
                        out=bcs[:], in_=recd[head, c].partition_broadcast(128))
                    return bcs

                state = {}

                def stage_b_wrap(state=state, stage_b=stage_b):
                    state["bcs"] = stage_b()

                def stage_c(head=head, c=c, o_ps=o_ps, state=state):
                    bcs = state.pop("bcs")
                    osb = ob_pool.tile([128, QC], BF16)
                    nc.vector.tensor_mul(osb[:], o_ps[:], bcs[:])
                    nc.sync.dma_start(
                        out=ot[head][:, QC * c:QC * (c + 1)], in_=osb[:])

                deferred.append(stage_b_wrap)
                deferred.append(lambda: None)   # spacer: let the bcast land
                deferred.append(stage_c)

    while deferred:
        deferred.pop(0)()


_CACHED_NC = None


def build_program():
    global _CACHED_NC
    if _CACHED_NC is not None:
        return _CACHED_NC
    nc = bacc.Bacc("TRN2", target_bir_lowering=False, debug=False,
                   num_devices=NCORES)
    qt = nc.dram_tensor("qt", [HEADS_PER_CORE, D, S], BF16,
                        kind="ExternalInput").ap()
    kt = nc.dram_tensor("kt", [PAIRS_PER_CORE, D, S], BF16,
                        kind="ExternalInput").ap()
    v = nc.dram_tensor("v", [PAIRS_PER_CORE, 128, NKT * D], BF16,
                       kind="ExternalInput").ap()
    recd = nc.dram_tensor("recd", [HEADS_PER_CORE, NQC, QC], F32,
                          kind="Internal").ap()
    ot = nc.dram_tensor("ot", [HEADS_PER_CORE, D, S], BF16,
                        kind="ExternalOutput").ap()
    with tile.TileContext(nc) as tc:
        emit_core_program(tc, qt, kt, v, recd, ot)
    nc.compile()
    _CACHED_NC = nc
    return nc


def shard_inputs(query, key, value):
    """Full inputs -> list of 8 per-core in_maps (host-side relayout + bf16
    cast; halves the HBM input traffic and keeps the PE in bf16)."""
    import ml_dtypes
    bf16 = ml_dtypes.bfloat16
    query = np.asarray(query, dtype=np.float32).astype(bf16)
    key = np.asarray(key, dtype=np.float32).astype(bf16)
    value = np.asarray(value, dtype=np.float32).astype(bf16)

    # Q: [S,B,HQ,D] -> [B*HKV, G, D, S]
    qtall = np.ascontiguousarray(
        query.reshape(S, B, HKV, G, D).transpose(1, 2, 3, 4, 0)
    ).reshape(NPAIRS, G, D, S)
    # K: [S,B,HKV,D] -> [B*HKV, D, S]
    ktall = np.ascontiguousarray(
        key.transpose(1, 2, 3, 0)).reshape(NPAIRS, D, S)
    # V: [S,B,HKV,D] -> [B*HKV, k_local=128, NKT*D]
    vall = np.ascontiguousarray(
        value.reshape(NKT, 128, B, HKV, D).transpose(2, 3, 1, 0, 4)
    ).reshape(NPAIRS, 128, NKT * D)

    in_maps = []
    for c in range(NCORES):
        p0 = PAIRS_PER_CORE * c
        p1 = p0 + PAIRS_PER_CORE
        in_maps.append({
            "qt": np.ascontiguousarray(qtall[p0:p1].reshape(HEADS_PER_CORE, D, S)),
            "kt": np.ascontiguousarray(ktall[p0:p1]),
            "v": np.ascontiguousarray(vall[p0:p1]),
        })
    return in_maps


def unshard_output(results):
    """8 per-core {'ot': [8, D, S]} -> full [S, B, HQ, D]."""
    ot = np.stack([np.asarray(r["ot"], dtype=np.float32) for r in results])
    ot = ot.reshape(B, HKV, G, D, S)                   # pairs major -> b, hkv
    out = np.ascontiguousarray(ot.transpose(4, 0, 1, 2, 3))  # [S,B,HKV,G,D]
    return out.reshape(S, B, HQ, D)


def kernel(query, key, value, _trace=False, _return_bkr=False):
    nc = build_program()
    in_maps = shard_inputs(query, key, value)
    bkr = bass_utils.run_bass_kernel_spmd(
        nc, in_maps, core_ids=list(range(NCORES)), trace=_trace)
    out = unshard_output(bkr.results)
    if _return_bkr:
        return out, bkr
    return out


if __name__ == "__main__":
    q = np.random.randn(S, B, HQ, D).astype(np.float32)
    k = np.random.randn(S, B, HKV, D).astype(np.float32)
    vv = np.random.randn(S, B, HKV, D).astype(np.float32)
    o = kernel(q, k, vv)
    print("out", o.shape, o.dtype, float(np.abs(o).max()))


# revision 12
# speedup vs baseline: 2.2955x; 1.4306x over previous
"""Causal GQA attention (S=2048, B=2, HQ=32, HKV=8, D=128) on 8 trn2 cores.

Sharding: the 16 (batch, kv-head) pairs are split 2 per core (data+head
parallel). Each pair carries group=4 query heads -> 8 attention heads/core.

Per head the kernel runs flash-attention style with the q-chunk loop OUTER
and the k-tile loop INNER:

  for q-chunk c (512 wide):                 # o accumulates in ONE psum bank
    for k-tile pair (t0, t1):               # 128-row k tiles, 2 at a time
      S^T(t0), S^T(t1) = (K_t Q_c^T) into a 2-bank psum pair
      P^T pair = exp(S^T pair)              # one 1024-wide ACT instruction
      tri-mask diagonal blocks (DVE)
      T += P^T tiles (DVE, bf16)            # cross-k-tile accumulation
      o += V_t^T P^T(t0), V_t^T P^T(t1)     # PE, accumulate in one bank
    den row = ones^T T                      # ONE 512-col matmul per chunk
    out_c = o * (1/den broadcast)           # DVE + DMA round trip

Everything on the PE runs bf16 (fp32/fp32r matmuls double-pump the array
and trip the power throttler). The one-matmul-per-chunk denominator (vs
one per k-tile) cuts PE streaming by ~1/3; the paired exp halves the
~185ns-per-instruction ACT bubble. Output is stored bf16 and upcast on
host (measured ~4e-3 rel err overall vs the 2e-2 gate).

Host side only re-lays-out data: Q/K are fed pre-transposed [d, s] bf16,
V as [k_local, ktile, d] bf16, and the returned out^T [d, s] bf16 is
transposed back and upcast.
"""

import numpy as np

import concourse.bass as bass
import concourse.mybir as mybir
import concourse.tile as tile
from concourse import bacc, bass_utils
from concourse.masks import make_identity, make_lower_triangular

S, B, HQ, HKV, D = 2048, 2, 32, 8, 128
G = HQ // HKV                      # 4 query heads per kv head
NCORES = 8
NPAIRS = B * HKV                   # 16 (batch, kv-head) pairs
PAIRS_PER_CORE = NPAIRS // NCORES  # 2
HEADS_PER_CORE = PAIRS_PER_CORE * G  # 8
SCALE = 1.0 / float(np.sqrt(D))
QC = 512                           # q-chunk (PSUM bank) width
NQC = S // QC                      # 4
KT = 128                           # k-tile (partition) width
NKT = S // KT                      # 16

F32 = mybir.dt.float32
BF16 = mybir.dt.bfloat16


def emit_core_program(tc, qt, kt, v, recd, ot):
    """Emit the per-core program.

    qt: [HEADS_PER_CORE, D, S] bf16   Q^T per head ([d, q])
    kt: [PAIRS_PER_CORE, D, S] bf16   K^T per pair ([d, k])
    v:  [PAIRS_PER_CORE, 128, NKT*D] bf16  V per pair ([k_local, kt, d])
    recd: [HEADS_PER_CORE, NQC, QC] f32 DRAM scratch for 1/sum rows
    ot: [HEADS_PER_CORE, D, S] bf16  out^T per head ([d, q])
    """
    from contextlib import ExitStack

    nc = tc.nc
    with ExitStack() as ctx:
        _emit_core_program(ctx, tc, nc, qt, kt, v, recd, ot)


def _emit_core_program(ctx, tc, nc, qt, kt, v, recd, ot):
    singles = ctx.enter_context(tc.tile_pool(name="singles", bufs=1))
    kv_pool = ctx.enter_context(tc.tile_pool(name="kv", bufs=2))
    q_pool = ctx.enter_context(tc.tile_pool(name="q", bufs=2))
    pp_pool = ctx.enter_context(tc.tile_pool(name="pp", bufs=4))
    t_pool = ctx.enter_context(tc.tile_pool(name="tt", bufs=2))
    ob_pool = ctx.enter_context(tc.tile_pool(name="ob", bufs=3))
    nrm_pool = ctx.enter_context(tc.tile_pool(name="nrm", bufs=3))
    ps_pair = ctx.enter_context(tc.tile_pool(name="ps_pair", bufs=2, space="PSUM"))
    ps_o = ctx.enter_context(tc.tile_pool(name="ps_o", bufs=3, space="PSUM"))
    ps_sum = ctx.enter_context(tc.tile_pool(name="ps_sum", bufs=1, space="PSUM"))

    # Constants
    # maskb[k, q] = -1e9 where q < k (causal-masked), 0 where q >= k. It is
    # injected into the S^T staging bank by an identity matmul with
    # start=True; the QK matmul then accumulates on top (start=False), so
    # exp(scale*(s - 1e9)) = 0 and the DVE never sits on the PE->ACT->PE
    # critical path.
    maskf = singles.tile([128, 128], F32)
    make_lower_triangular(nc, maskf[:], val=-1e9, diag=False)
    maskb = singles.tile([128, 128], BF16)
    nc.scalar.copy(out=maskb[:], in_=maskf[:])
    identf = singles.tile([128, 128], F32)
    make_identity(nc, identf[:])
    identb = singles.tile([128, 128], BF16)
    nc.scalar.copy(out=identb[:], in_=identf[:])
    onesc = singles.tile([128, 1], BF16)   # ones column (sum-over-k lhsT)
    nc.vector.memset(onesc[:], 1.0)

    # Deferred normalization stages: one closure is popped and emitted at
    # the top of each k-tile-pair iteration, so the slow DMA round trips
    # (recip row -> DRAM -> partition-broadcast) never head-of-line block
    # the DVE queue that feeds T accumulation.
    deferred = []

    def pop_deferred():
        if deferred:
            deferred.pop(0)()

    exp = mybir.ActivationFunctionType.Exp

    for pair in range(PAIRS_PER_CORE):
        kt_sb = kv_pool.tile([D, S], BF16, tag="kt")
        nc.sync.dma_start(out=kt_sb[:], in_=kt[pair])
        v_sb = kv_pool.tile([128, NKT * D], BF16, tag="v")
        nc.gpsimd.dma_start(out=v_sb[:], in_=v[pair])

        for g in range(G):
            head = pair * G + g
            q_sb = q_pool.tile([D, S], BF16)
            nc.sync.dma_start(out=q_sb[:], in_=qt[head])

            for c in range(NQC):
                ntiles = 4 * c + 4
                npairs = ntiles // 2
                qs = q_sb[:, QC * c:QC * (c + 1)]
                o_ps = ps_o.tile([128, QC], F32, tag="o", name=f"o_{head}_{c}")
                tt = t_pool.tile([128, QC], BF16, tag="T", name=f"T_{head}_{c}")

                for j in range(npairs):
                    pop_deferred()
                    t0, t1 = 2 * j, 2 * j + 1
                    o0 = max(0, KT * t0 - QC * c)
                    o1 = max(0, KT * t1 - QC * c)
                    sp = ps_pair.tile([128, 2 * QC], F32, tag="spair")
                    pp = pp_pool.tile([128, 2 * QC], BF16, tag="pp")

                    # S^T = (Q K^T)^T for both k-tiles of the pair. On the
                    # diagonal tiles an identity matmul first deposits the
                    # -1e9 causal mask into the staging bank (start=True
                    # clears the bank + sets has_written on the block), and
                    # the QK matmul accumulates on top of it.
                    for tti, oo, base in ((t0, o0, 0), (t1, o1, QC)):
                        diag = tti >= 4 * c
                        if diag:
                            nc.tensor.matmul(
                                out=sp[:, base + oo:base + oo + KT],
                                lhsT=identb[:], rhs=maskb[:],
                                start=True, stop=False)
                        nc.tensor.matmul(
                            out=sp[:, base + oo:base + QC],
                            lhsT=kt_sb[:, KT * tti:KT * (tti + 1)],
                            rhs=qs[:, oo:QC], start=not diag, stop=True)

                    # one wide exp over the pair (cols [QC+o0 : QC+o1] are
                    # junk from the staging bank; nothing consumes them)
                    nc.scalar.activation(
                        pp[:, o0:2 * QC], sp[:, o0:2 * QC], exp, scale=SCALE)

                    # T accumulation (bf16, 2x DVE mode)
                    if j == 0:
                        nc.vector.tensor_copy(tt[:, :], pp[:, 0:QC])
                    else:
                        nc.vector.tensor_add(
                            tt[:, o0:QC], tt[:, o0:QC], pp[:, o0:QC])
                    nc.vector.tensor_add(
                        tt[:, o1:QC], tt[:, o1:QC], pp[:, QC + o1:2 * QC])

                    # out^T accumulation, V stationary
                    nc.tensor.matmul(
                        out=o_ps[:, o0:QC],
                        lhsT=v_sb[:, D * t0:D * (t0 + 1)],
                        rhs=pp[:, o0:QC],
                        start=(j == 0), stop=False)
                    nc.tensor.matmul(
                        out=o_ps[:, o1:QC],
                        lhsT=v_sb[:, D * t1:D * (t1 + 1)],
                        rhs=pp[:, QC + o1:2 * QC],
                        start=False, stop=(j == npairs - 1))

                # denominator: one 512-col stream over the accumulated T
                sum_ps = ps_sum.tile([128, QC], F32, tag="sum")
                nc.tensor.matmul(
                    out=sum_ps[0:1, :], lhsT=onesc[:], rhs=tt[:, :],
                    start=True, stop=True)

                # Normalization tail, deferred + spaced so the DMA round
                # trips never head-of-line block the DVE queue:
                #   A: pull the sums row out of PSUM, DMA-reshape to
                #      [128, 4] so the reciprocal runs 128 lanes wide
                #      (a [1, 512] reciprocal measures ~4us on the DVE!)
                #   B: reciprocal, park in DRAM, partition-broadcast back
                #   C: multiply into the out^T chunk and store
                state = {}

                def stage_a(state=state, sum_ps=sum_ps):
                    sr = nrm_pool.tile([1, QC], F32, tag="sumrow")
                    nc.vector.tensor_copy(sr[:], sum_ps[0:1, :])
                    srec = nrm_pool.tile([128, NQC], F32, tag="srec")
                    nc.sync.dma_start(out=srec[:], in_=sr[:])
                    state["srec"] = srec

                def stage_b(head=head, c=c, state=state):
                    srec2 = nrm_pool.tile([128, NQC], F32, tag="srec2")
                    nc.vector.reciprocal(out=srec2[:], in_=state.pop("srec")[:])
                    nc.sync.dma_start(out=recd[head, c], in_=srec2[:])
                    bcs = nrm_pool.tile([128, QC], F32, tag="bc")
                    nc.sync.dma_start(
                        out=bcs[:], in_=recd[head, c].partition_broadcast(128))
                    state["bcs"] = bcs

                def stage_c(head=head, c=c, o_ps=o_ps, state=state):
                    bcs = state.pop("bcs")
                    osb = ob_pool.tile([128, QC], BF16)
                    nc.vector.tensor_mul(osb[:], o_ps[:], bcs[:])
                    nc.sync.dma_start(
                        out=ot[head][:, QC * c:QC * (c + 1)], in_=osb[:])

                deferred.append(stage_a)
                deferred.append(lambda: None)   # let the reshape DMA land
                deferred.append(stage_b)
                deferred.append(lambda: None)   # let the broadcast land
                deferred.append(stage_c)

    while deferred:
        deferred.pop(0)()


_CACHED_NC = None


def build_program():
    global _CACHED_NC
    if _CACHED_NC is not None:
        return _CACHED_NC
    nc = bacc.Bacc("TRN2", target_bir_lowering=False, debug=False,
                   num_devices=NCORES)
    qt = nc.dram_tensor("qt", [HEADS_PER_CORE, D, S], BF16,
                        kind="ExternalInput").ap()
    kt = nc.dram_tensor("kt", [PAIRS_PER_CORE, D, S], BF16,
                        kind="ExternalInput").ap()
    v = nc.dram_tensor("v", [PAIRS_PER_CORE, 128, NKT * D], BF16,
                       kind="ExternalInput").ap()
    recd = nc.dram_tensor("recd", [HEADS_PER_CORE, NQC, QC], F32,
                          kind="Internal").ap()
    ot = nc.dram_tensor("ot", [HEADS_PER_CORE, D, S], BF16,
                        kind="ExternalOutput").ap()
    with tile.TileContext(nc) as tc:
        emit_core_program(tc, qt, kt, v, recd, ot)
    nc.compile()
    _CACHED_NC = nc
    return nc


def shard_inputs(query, key, value):
    """Full inputs -> list of 8 per-core in_maps (host-side relayout + bf16
    cast; halves the HBM input traffic and keeps the PE in bf16)."""
    import ml_dtypes
    bf16 = ml_dtypes.bfloat16
    query = np.asarray(query, dtype=np.float32).astype(bf16)
    key = np.asarray(key, dtype=np.float32).astype(bf16)
    value = np.asarray(value, dtype=np.float32).astype(bf16)

    # Q: [S,B,HQ,D] -> [B*HKV, G, D, S]
    qtall = np.ascontiguousarray(
        query.reshape(S, B, HKV, G, D).transpose(1, 2, 3, 4, 0)
    ).reshape(NPAIRS, G, D, S)
    # K: [S,B,HKV,D] -> [B*HKV, D, S]
    ktall = np.ascontiguousarray(
        key.transpose(1, 2, 3, 0)).reshape(NPAIRS, D, S)
    # V: [S,B,HKV,D] -> [B*HKV, k_local=128, NKT*D]
    vall = np.ascontiguousarray(
        value.reshape(NKT, 128, B, HKV, D).transpose(2, 3, 1, 0, 4)
    ).reshape(NPAIRS, 128, NKT * D)

    in_maps = []
    for c in range(NCORES):
        p0 = PAIRS_PER_CORE * c
        p1 = p0 + PAIRS_PER_CORE
        in_maps.append({
            "qt": np.ascontiguousarray(qtall[p0:p1].reshape(HEADS_PER_CORE, D, S)),
            "kt": np.ascontiguousarray(ktall[p0:p1]),
            "v": np.ascontiguousarray(vall[p0:p1]),
        })
    return in_maps


def unshard_output(results):
    """8 per-core {'ot': [8, D, S]} -> full [S, B, HQ, D]."""
    ot = np.stack([np.asarray(r["ot"], dtype=np.float32) for r in results])
    ot = ot.reshape(B, HKV, G, D, S)                   # pairs major -> b, hkv
    out = np.ascontiguousarray(ot.transpose(4, 0, 1, 2, 3))  # [S,B,HKV,G,D]
    return out.reshape(S, B, HQ, D)


def kernel(query, key, value, _trace=False, _return_bkr=False):
    nc = build_program()
    in_maps = shard_inputs(query, key, value)
    bkr = bass_utils.run_bass_kernel_spmd(
        nc, in_maps, core_ids=list(range(NCORES)), trace=_trace)
    out = unshard_output(bkr.results)
    if _return_bkr:
        return out, bkr
    return out


if __name__ == "__main__":
    q = np.random.randn(S, B, HQ, D).astype(np.float32)
    k = np.random.randn(S, B, HKV, D).astype(np.float32)
    vv = np.random.randn(S, B, HKV, D).astype(np.float32)
    o = kernel(q, k, vv)
    print("out", o.shape, o.dtype, float(np.abs(o).max()))


# revision 14
# speedup vs baseline: 2.3141x; 1.0081x over previous
"""Causal GQA attention (S=2048, B=2, HQ=32, HKV=8, D=128) on 8 trn2 cores.

Sharding: the 16 (batch, kv-head) pairs are split 2 per core (data+head
parallel). Each pair carries group=4 query heads -> 8 attention heads/core.

Per head the kernel runs flash-attention style with the q-chunk loop OUTER
and the k-tile loop INNER:

  for q-chunk c (512 wide):                 # o accumulates in ONE psum bank
    for k-tile pair (t0, t1):               # 128-row k tiles, 2 at a time
      S^T(t0), S^T(t1) = (K_t Q_c^T) into a 2-bank psum pair
      P^T pair = exp(S^T pair)              # one 1024-wide ACT instruction
      tri-mask diagonal blocks (DVE)
      T += P^T tiles (DVE, bf16)            # cross-k-tile accumulation
      o += V_t^T P^T(t0), V_t^T P^T(t1)     # PE, accumulate in one bank
    den row = ones^T T                      # ONE 512-col matmul per chunk
    out_c = o * (1/den broadcast)           # DVE + DMA round trip

Everything on the PE runs bf16 (fp32/fp32r matmuls double-pump the array
and trip the power throttler). The one-matmul-per-chunk denominator (vs
one per k-tile) cuts PE streaming by ~1/3; the paired exp halves the
~185ns-per-instruction ACT bubble. Output is stored bf16 and upcast on
host (measured ~4e-3 rel err overall vs the 2e-2 gate).

Host side only re-lays-out data: Q/K are fed pre-transposed [d, s] bf16,
V as [k_local, ktile, d] bf16, and the returned out^T [d, s] bf16 is
transposed back and upcast.
"""

import numpy as np

import concourse.bass as bass
import concourse.mybir as mybir
import concourse.tile as tile
from concourse import bacc, bass_utils
from concourse.masks import make_identity, make_lower_triangular

S, B, HQ, HKV, D = 2048, 2, 32, 8, 128
G = HQ // HKV                      # 4 query heads per kv head
NCORES = 8
NPAIRS = B * HKV                   # 16 (batch, kv-head) pairs
PAIRS_PER_CORE = NPAIRS // NCORES  # 2
HEADS_PER_CORE = PAIRS_PER_CORE * G  # 8
SCALE = 1.0 / float(np.sqrt(D))
QC = 512                           # q-chunk (PSUM bank) width
NQC = S // QC                      # 4
KT = 128                           # k-tile (partition) width
NKT = S // KT                      # 16

F32 = mybir.dt.float32
BF16 = mybir.dt.bfloat16


def emit_core_program(tc, qt, kt, v, recd, ot):
    """Emit the per-core program.

    qt: [HEADS_PER_CORE, D, S] bf16   Q^T per head ([d, q])
    kt: [PAIRS_PER_CORE, D, S] bf16   K^T per pair ([d, k])
    v:  [PAIRS_PER_CORE, 128, NKT*D] bf16  V per pair ([k_local, kt, d])
    recd: [HEADS_PER_CORE, NQC, QC] f32 DRAM scratch for 1/sum rows
    ot: [HEADS_PER_CORE, D, S] bf16  out^T per head ([d, q])
    """
    from contextlib import ExitStack

    nc = tc.nc
    with ExitStack() as ctx:
        _emit_core_program(ctx, tc, nc, qt, kt, v, recd, ot)


def _emit_core_program(ctx, tc, nc, qt, kt, v, recd, ot):
    singles = ctx.enter_context(tc.tile_pool(name="singles", bufs=1))
    kv_pool = ctx.enter_context(tc.tile_pool(name="kv", bufs=2))
    q_pool = ctx.enter_context(tc.tile_pool(name="q", bufs=2))
    pp_pool = ctx.enter_context(tc.tile_pool(name="pp", bufs=6))
    t_pool = ctx.enter_context(tc.tile_pool(name="tt", bufs=3))
    ob_pool = ctx.enter_context(tc.tile_pool(name="ob", bufs=3))
    nrm_pool = ctx.enter_context(tc.tile_pool(name="nrm", bufs=3))
    ps_pair = ctx.enter_context(tc.tile_pool(name="ps_pair", bufs=2, space="PSUM"))
    ps_o = ctx.enter_context(tc.tile_pool(name="ps_o", bufs=3, space="PSUM"))
    ps_sum = ctx.enter_context(tc.tile_pool(name="ps_sum", bufs=1, space="PSUM"))

    # Constants
    # maskb[k, q] = -1e9 where q < k (causal-masked), 0 where q >= k. It is
    # injected into the S^T staging bank by an identity matmul with
    # start=True; the QK matmul then accumulates on top (start=False), so
    # exp(scale*(s - 1e9)) = 0 and the DVE never sits on the PE->ACT->PE
    # critical path.
    maskf = singles.tile([128, 128], F32)
    make_lower_triangular(nc, maskf[:], val=-1e9, diag=False)
    maskb = singles.tile([128, 128], BF16)
    nc.scalar.copy(out=maskb[:], in_=maskf[:])
    identf = singles.tile([128, 128], F32)
    make_identity(nc, identf[:])
    identb = singles.tile([128, 128], BF16)
    nc.scalar.copy(out=identb[:], in_=identf[:])
    onesc = singles.tile([128, 1], BF16)   # ones column (sum-over-k lhsT)
    nc.vector.memset(onesc[:], 1.0)

    # Deferred normalization stages: one closure is popped and emitted at
    # the top of each k-tile-pair iteration, so the slow DMA round trips
    # (recip row -> DRAM -> partition-broadcast) never head-of-line block
    # the DVE queue that feeds T accumulation.
    deferred = []

    def pop_deferred():
        if deferred:
            deferred.pop(0)()

    exp = mybir.ActivationFunctionType.Exp

    for pair in range(PAIRS_PER_CORE):
        kt_sb = kv_pool.tile([D, S], BF16, tag="kt")
        nc.sync.dma_start(out=kt_sb[:], in_=kt[pair])
        v_sb = kv_pool.tile([128, NKT * D], BF16, tag="v")
        nc.gpsimd.dma_start(out=v_sb[:], in_=v[pair])

        for g in range(G):
            head = pair * G + g
            q_sb = q_pool.tile([D, S], BF16)
            nc.sync.dma_start(out=q_sb[:], in_=qt[head])

            for c in range(NQC):
                ntiles = 4 * c + 4
                npairs = ntiles // 2
                qs = q_sb[:, QC * c:QC * (c + 1)]
                o_ps = ps_o.tile([128, QC], F32, tag="o", name=f"o_{head}_{c}")
                tt = t_pool.tile([128, QC], BF16, tag="T", name=f"T_{head}_{c}")

                for j in range(npairs):
                    pop_deferred()
                    t0, t1 = 2 * j, 2 * j + 1
                    o0 = max(0, KT * t0 - QC * c)
                    o1 = max(0, KT * t1 - QC * c)
                    sp = ps_pair.tile([128, 2 * QC], F32, tag="spair")
                    pp = pp_pool.tile([128, 2 * QC], BF16, tag="pp")

                    # S^T = (Q K^T)^T for both k-tiles of the pair. On the
                    # diagonal tiles an identity matmul first deposits the
                    # -1e9 causal mask into the staging bank (start=True
                    # clears the bank + sets has_written on the block), and
                    # the QK matmul accumulates on top of it.
                    for tti, oo, base in ((t0, o0, 0), (t1, o1, QC)):
                        diag = tti >= 4 * c
                        if diag:
                            nc.tensor.matmul(
                                out=sp[:, base + oo:base + oo + KT],
                                lhsT=identb[:], rhs=maskb[:],
                                start=True, stop=False)
                        nc.tensor.matmul(
                            out=sp[:, base + oo:base + QC],
                            lhsT=kt_sb[:, KT * tti:KT * (tti + 1)],
                            rhs=qs[:, oo:QC], start=not diag, stop=True)

                    # one wide exp over the pair (cols [QC+o0 : QC+o1] are
                    # junk from the staging bank; nothing consumes them)
                    nc.scalar.activation(
                        pp[:, o0:2 * QC], sp[:, o0:2 * QC], exp, scale=SCALE)

                    # T accumulation (bf16, 2x DVE mode). The LAST pair of
                    # the chunk skips T: its denominator contribution goes
                    # through direct ones-matmuls on the exp output below,
                    # so the chunk tail never waits on the DVE chain.
                    last = j == npairs - 1
                    if j == 0:
                        nc.vector.tensor_copy(tt[:, :], pp[:, 0:QC])
                    elif not last:
                        nc.vector.tensor_add(
                            tt[:, o0:QC], tt[:, o0:QC], pp[:, o0:QC])
                    if not last:
                        nc.vector.tensor_add(
                            tt[:, o1:QC], tt[:, o1:QC], pp[:, QC + o1:2 * QC])

                    if last:
                        # denominator: one 512-col stream over T (pairs
                        # 0..n-2) + the last pair's slices streamed direct
                        sum_ps = ps_sum.tile([128, QC], F32, tag="sum")
                        nc.tensor.matmul(
                            out=sum_ps[0:1, :], lhsT=onesc[:], rhs=tt[:, :],
                            start=True, stop=False)
                        nc.tensor.matmul(
                            out=sum_ps[0:1, o0:QC], lhsT=onesc[:],
                            rhs=pp[:, o0:QC], start=False, stop=False)
                        nc.tensor.matmul(
                            out=sum_ps[0:1, o1:QC], lhsT=onesc[:],
                            rhs=pp[:, QC + o1:2 * QC], start=False, stop=True)

                    # out^T accumulation, V stationary
                    nc.tensor.matmul(
                        out=o_ps[:, o0:QC],
                        lhsT=v_sb[:, D * t0:D * (t0 + 1)],
                        rhs=pp[:, o0:QC],
                        start=(j == 0), stop=False)
                    nc.tensor.matmul(
                        out=o_ps[:, o1:QC],
                        lhsT=v_sb[:, D * t1:D * (t1 + 1)],
                        rhs=pp[:, QC + o1:2 * QC],
                        start=False, stop=(j == npairs - 1))

                # Normalization tail, deferred + spaced so the DMA round
                # trips never head-of-line block the DVE queue:
                #   A: pull the sums row out of PSUM, DMA-reshape to
                #      [128, 4] so the reciprocal runs 128 lanes wide
                #      (a [1, 512] reciprocal measures ~4us on the DVE!)
                #   B: reciprocal, park in DRAM, partition-broadcast back
                #   C: multiply into the out^T chunk and store
                state = {}

                def stage_a(state=state, sum_ps=sum_ps):
                    sr = nrm_pool.tile([1, QC], F32, tag="sumrow")
                    nc.vector.tensor_copy(sr[:], sum_ps[0:1, :])
                    srec = nrm_pool.tile([128, NQC], F32, tag="srec")
                    nc.sync.dma_start(out=srec[:], in_=sr[:])
                    state["srec"] = srec

                def stage_b(head=head, c=c, state=state):
                    srec2 = nrm_pool.tile([128, NQC], F32, tag="srec2")
                    nc.vector.reciprocal(out=srec2[:], in_=state.pop("srec")[:])
                    nc.sync.dma_start(out=recd[head, c], in_=srec2[:])
                    bcs = nrm_pool.tile([128, QC], F32, tag="bc")
                    nc.sync.dma_start(
                        out=bcs[:], in_=recd[head, c].partition_broadcast(128))
                    state["bcs"] = bcs

                def stage_c(head=head, c=c, o_ps=o_ps, state=state):
                    bcs = state.pop("bcs")
                    osb = ob_pool.tile([128, QC], BF16)
                    nc.vector.tensor_mul(osb[:], o_ps[:], bcs[:])
                    nc.sync.dma_start(
                        out=ot[head][:, QC * c:QC * (c + 1)], in_=osb[:])

                deferred.append(stage_a)
                deferred.append(lambda: None)   # let the reshape DMA land
                deferred.append(stage_b)
                deferred.append(lambda: None)   # let the broadcast land
                deferred.append(stage_c)

    while deferred:
        deferred.pop(0)()


_CACHED_NC = None


def build_program():
    global _CACHED_NC
    if _CACHED_NC is not None:
        return _CACHED_NC
    nc = bacc.Bacc("TRN2", target_bir_lowering=False, debug=False,
                   num_devices=NCORES)
    qt = nc.dram_tensor("qt", [HEADS_PER_CORE, D, S], BF16,
                        kind="ExternalInput").ap()
    kt = nc.dram_tensor("kt", [PAIRS_PER_CORE, D, S], BF16,
                        kind="ExternalInput").ap()
    v = nc.dram_tensor("v", [PAIRS_PER_CORE, 128, NKT * D], BF16,
                       kind="ExternalInput").ap()
    recd = nc.dram_tensor("recd", [HEADS_PER_CORE, NQC, QC], F32,
                          kind="Internal").ap()
    ot = nc.dram_tensor("ot", [HEADS_PER_CORE, D, S], BF16,
                        kind="ExternalOutput").ap()
    with tile.TileContext(nc) as tc:
        emit_core_program(tc, qt, kt, v, recd, ot)
    nc.compile()
    _CACHED_NC = nc
    return nc


def shard_inputs(query, key, value):
    """Full inputs -> list of 8 per-core in_maps (host-side relayout + bf16
    cast; halves the HBM input traffic and keeps the PE in bf16)."""
    import ml_dtypes
    bf16 = ml_dtypes.bfloat16
    query = np.asarray(query, dtype=np.float32).astype(bf16)
    key = np.asarray(key, dtype=np.float32).astype(bf16)
    value = np.asarray(value, dtype=np.float32).astype(bf16)

    # Q: [S,B,HQ,D] -> [B*HKV, G, D, S]
    qtall = np.ascontiguousarray(
        query.reshape(S, B, HKV, G, D).transpose(1, 2, 3, 4, 0)
    ).reshape(NPAIRS, G, D, S)
    # K: [S,B,HKV,D] -> [B*HKV, D, S]
    ktall = np.ascontiguousarray(
        key.transpose(1, 2, 3, 0)).reshape(NPAIRS, D, S)
    # V: [S,B,HKV,D] -> [B*HKV, k_local=128, NKT*D]
    vall = np.ascontiguousarray(
        value.reshape(NKT, 128, B, HKV, D).transpose(2, 3, 1, 0, 4)
    ).reshape(NPAIRS, 128, NKT * D)

    in_maps = []
    for c in range(NCORES):
        p0 = PAIRS_PER_CORE * c
        p1 = p0 + PAIRS_PER_CORE
        in_maps.append({
            "qt": np.ascontiguousarray(qtall[p0:p1].reshape(HEADS_PER_CORE, D, S)),
            "kt": np.ascontiguousarray(ktall[p0:p1]),
            "v": np.ascontiguousarray(vall[p0:p1]),
        })
    return in_maps


def unshard_output(results):
    """8 per-core {'ot': [8, D, S]} -> full [S, B, HQ, D]."""
    ot = np.stack([np.asarray(r["ot"], dtype=np.float32) for r in results])
    ot = ot.reshape(B, HKV, G, D, S)                   # pairs major -> b, hkv
    out = np.ascontiguousarray(ot.transpose(4, 0, 1, 2, 3))  # [S,B,HKV,G,D]
    return out.reshape(S, B, HQ, D)


def kernel(query, key, value, _trace=False, _return_bkr=False):
    nc = build_program()
    in_maps = shard_inputs(query, key, value)
    bkr = bass_utils.run_bass_kernel_spmd(
        nc, in_maps, core_ids=list(range(NCORES)), trace=_trace)
    out = unshard_output(bkr.results)
    if _return_bkr:
        return out, bkr
    return out


if __name__ == "__main__":
    q = np.random.randn(S, B, HQ, D).astype(np.float32)
    k = np.random.randn(S, B, HKV, D).astype(np.float32)
    vv = np.random.randn(S, B, HKV, D).astype(np.float32)
    o = kernel(q, k, vv)
    print("out", o.shape, o.dtype, float(np.abs(o).max()))
